# revision 4
# baseline (speedup 1.0000x reference)
"""Trainium2 Bass kernel for hash-indexed per-pixel conv (nn_ABC_2D).

Reference computation:
    patches[b,p,c] = x.reshape(B,-1)[b, hash_idx[p,c]]        # [B,P,CK] gather
    out[b,k,p]     = sum_c weights[p,k,c] * patches[b,p,c]   # per-pixel matmul

Sharding: pixels P=16384 split across 8 cores (2048 each); each core streams
its weight slab and patch slab and computes all B=8 batches.

Device kernel structure (per core, per 256-pixel tile):
  - weights arrive pre-transposed [c, p, k] and patches [c, p, b] (c on
    partitions), so TensorE contracts over c directly: stationary =
    w[c,(8p,16k)] (128 cols), moving = patches[c,(8p,8b)] (64 cols), PSUM
    [(p,k), (p',b)]; 2 matmuls per 8-pixel group (c split 128+16).
  - DMA layout is engineered around the 16-SDMA-engine port map: each
    engine owns 8 fixed partitions, so every transfer must span a balanced
    partition set.  Main chunks are [128, N] (all 16 engines).  The
    16-channel tails are parked in persistent holders at partition base
    32*(t%4), rotating across tiles so the 4-engine quads take turns; the
    tail matmuls use the PE tile_position row-group mechanism (stationary
    rows at base 32q, rhs partitions to match).
  - weight DMAs issue from SyncE's HWDGE ring, patch DMAs from ScalarE's,
    so the streams drain concurrently.
  - the diagonal blocks (p'==p) are extracted with a mask multiply +
    tensor_reduce over p' (engine APs cannot start at 16-aligned
    partitions, so no sub-32 partition slicing is possible).  extract="act"
    has ScalarE copy PSUM->SBUF bf16 first so the DVE multiply runs in
    2x_1P mode and the PSUM bank frees earlier.
  - weights/patches travel in bf16 (host converts; ~halves HBM traffic).

The hash gather itself is done on the host: every device-side fine-grained
gather path was measured or ruled out on silicon (indirect_dma_start
consumes one index per partition per instruction; dma_gather needs 256-byte
elements; GpSimd ap_gather tables cap at 128KB per 16-partition group).

Output is returned per-core as [128, 2048] = [(p%8,k), (tile, grp, b)] and
reassembled/permuted on the host.
"""
import numpy as np

B, C, H, W = 8, 16, 128, 128
P = H * W            # 16384
KN = 16
CK = C * 9           # 144
NCORES = 8
PPC = P // NCORES    # 2048 pixels per core
C0 = 128
C1 = CK - C0         # 16
GRP = 8              # pixels per matmul group (8px*16k = 128 stationary cols)
TILE_P = 256         # pixels per tile
GRPS_PER_TILE = TILE_P // GRP      # 32
NTILES = PPC // TILE_P             # 8

EXTRACT = "act"      # "dve" or "act"

_CACHE = {}


def build(reps=1, extract=EXTRACT):
    from concourse import bacc, bass, mybir, tile

    wire_dt = mybir.dt.bfloat16
    msk_dt = mybir.dt.bfloat16 if extract == "act" else mybir.dt.float32
    nc = bacc.Bacc(None)
    ppc = NTILES * TILE_P
    w0 = nc.declare_dram_parameter("w0", [C0, ppc * KN], wire_dt, isOutput=False)
    w1 = nc.declare_dram_parameter("w1", [C1, ppc * KN], wire_dt, isOutput=False)
    p0 = nc.declare_dram_parameter("p0", [C0, ppc * B], wire_dt, isOutput=False)
    p1 = nc.declare_dram_parameter("p1", [C1, ppc * B], wire_dt, isOutput=False)
    msk = nc.declare_dram_parameter("msk", [128, GRPS_PER_TILE * GRP * B],
                                    msk_dt, isOutput=False)
    out = nc.declare_dram_parameter("out", [128, ppc], mybir.dt.float32, isOutput=True)

    with tile.TileContext(nc) as tc:
        with (
            tc.tile_pool(name="wp", bufs=3) as wp,
            tc.tile_pool(name="gp", bufs=3) as gp,
            tc.tile_pool(name="th", bufs=2) as th,
            tc.tile_pool(name="sp", bufs=3) as sp,
            tc.tile_pool(name="op", bufs=1) as op,
            tc.tile_pool(name="ps", bufs=2, space="PSUM") as ps,
        ):
            o_sb = op.tile([128, ppc], mybir.dt.float32)
            msk_sb = op.tile([128, GRPS_PER_TILE * GRP * B], msk_dt)
            nc.sync.dma_start(out=msk_sb[:], in_=msk[:])

            def body(_iv=None):
                # Persistent per-iteration holders for the 16-channel tails:
                # tile t lives at partition base 32*(t%3) (PE quadrant 3 is
                # unusable), column slot t//3.
                w1h = th.tile([128, 3 * TILE_P * KN], wire_dt, tag="w1h")
                g1h = th.tile([128, 3 * TILE_P * B], wire_dt, tag="g1h")
                for t in range(NTILES):
                    q = 32 * (t % 3)
                    h = t // 3
                    wt0 = wp.tile([C0, TILE_P * KN], wire_dt, tag="w0")
                    nc.sync.dma_start(
                        out=wt0[:], in_=w0[:, t * TILE_P * KN:(t + 1) * TILE_P * KN])
                    nc.sync.dma_start(
                        out=w1h[q:q + C1,
                                h * TILE_P * KN:(h + 1) * TILE_P * KN],
                        in_=w1[:, t * TILE_P * KN:(t + 1) * TILE_P * KN])
                    g0 = gp.tile([C0, TILE_P, B], wire_dt, tag="g0")
                    nc.scalar.dma_start(
                        out=g0[:].rearrange("c p b -> c (p b)"),
                        in_=p0[:, t * TILE_P * B:(t + 1) * TILE_P * B])
                    nc.scalar.dma_start(
                        out=g1h[q:q + C1, h * TILE_P * B:(h + 1) * TILE_P * B],
                        in_=p1[:, t * TILE_P * B:(t + 1) * TILE_P * B])

                    ps_t = ps.tile([128, GRPS_PER_TILE * GRP * B], mybir.dt.float32,
                                   space="PSUM", tag="acc")
                    for g in range(GRPS_PER_TILE):
                        pix = g * GRP
                        nc.tensor.matmul(
                            out=ps_t[:, g * GRP * B:(g + 1) * GRP * B],
                            lhsT=wt0[:, pix * KN:(pix + GRP) * KN],
                            rhs=g0[:, pix:pix + GRP, :],
                            start=True, stop=False)
                        nc.tensor.matmul(
                            out=ps_t[:, g * GRP * B:(g + 1) * GRP * B],
                            lhsT=w1h[q:q + C1,
                                     h * TILE_P * KN + pix * KN:
                                     h * TILE_P * KN + (pix + GRP) * KN],
                            rhs=g1h[q:q + C1,
                                    h * TILE_P * B + pix * B:
                                    h * TILE_P * B + (pix + GRP) * B].rearrange(
                                        "c (p b) -> c p b", p=GRP),
                            start=False, stop=True)
                    # Diagonal extraction without sub-32 partition slicing:
                    # mask out off-diagonal pixel columns, then reduce over p'.
                    if extract == "act":
                        c_t = sp.tile([128, GRPS_PER_TILE * GRP * B],
                                      mybir.dt.bfloat16, tag="ct")
                        nc.scalar.copy(out=c_t[:], in_=ps_t[:])
                        mul_in = c_t
                        s_dt = mybir.dt.bfloat16
                    else:
                        mul_in = ps_t
                        s_dt = mybir.dt.float32
                    s_t = sp.tile([128, GRPS_PER_TILE * GRP * B], s_dt, tag="st")
                    nc.vector.tensor_tensor(
                        out=s_t[:], in0=mul_in[:], in1=msk_sb[:],
                        op=mybir.AluOpType.mult)
                    nc.vector.tensor_reduce(
                        out=o_sb[:, t * TILE_P:(t + 1) * TILE_P].rearrange(
                            "q (G b) -> q G b", G=GRPS_PER_TILE, b=B),
                        in_=s_t[:].rearrange(
                            "q (G p b) -> q G b p", G=GRPS_PER_TILE, p=GRP, b=B),
                        axis=mybir.AxisListType.X,
                        op=mybir.AluOpType.add)

            if reps == 1:
                body()
            else:
                with tc.For_i(0, reps, 1) as _i:
                    body(_i)
            nc.sync.dma_start(out=out[:], in_=o_sb[:])
    nc.finalize()
    return nc


def make_mask(extract=EXTRACT):
    """mask[(pl,k), (G,p',b)] = 1 when p' == pl."""
    m = np.zeros((GRP, KN, GRPS_PER_TILE, GRP, B), dtype=np.float32)
    for pl in range(GRP):
        m[pl, :, :, pl, :] = 1.0
    m = m.reshape(128, GRPS_PER_TILE * GRP * B)
    if extract == "act":
        import ml_dtypes
        m = m.astype(ml_dtypes.bfloat16)
    return m


def prep_host_inputs(x, weights, hash_idx, extract=EXTRACT):
    """Per-core input maps (patches gathered in numpy, all wires bf16)."""
    import ml_dtypes
    B_, Cc, Hh, Ww = x.shape
    flat = np.asarray(x).reshape(B_, -1)
    msk = make_mask(extract)
    conv = lambda a: a.astype(ml_dtypes.bfloat16)
    in_maps = []
    for c in range(NCORES):
        sl = slice(c * PPC, (c + 1) * PPC)
        w_t = np.ascontiguousarray(np.asarray(weights)[sl].transpose(2, 0, 1))  # [CK,PPC,KN]
        idx_t = np.asarray(hash_idx)[sl].T                                       # [CK,PPC]
        pat = flat[:, idx_t].transpose(1, 2, 0)                                  # [CK,PPC,B]
        pat = np.ascontiguousarray(pat, dtype=np.float32)
        in_maps.append({
            "w0": conv(w_t[:C0].reshape(C0, PPC * KN)),
            "w1": conv(w_t[C0:].reshape(C1, PPC * KN)),
            "p0": conv(pat[:C0].reshape(C0, PPC * B)),
            "p1": conv(pat[C0:].reshape(C1, PPC * B)),
            "msk": msk,
        })
    return in_maps


def assemble(results, ppc=PPC):
    """Per-core o[(pl,k), (t,G,b)] -> full [B, KN, P]; p = t*256 + G*8 + pl."""
    outs = []
    for r in results:
        o = r["out"].reshape(GRP, KN, ppc // TILE_P, GRPS_PER_TILE, B)
        o = o.transpose(4, 1, 2, 3, 0).reshape(B, KN, ppc)
        outs.append(o)
    return np.concatenate(outs, axis=2)


def kernel(x, weights, hash_idx):
    from concourse.bass_utils import run_bass_kernel_spmd

    if "nc" not in _CACHE:
        _CACHE["nc"] = build()
    nc = _CACHE["nc"]
    in_maps = prep_host_inputs(np.asarray(x), np.asarray(weights),
                               np.asarray(hash_idx))
    res = run_bass_kernel_spmd(nc, in_maps, list(range(NCORES)))
    return assemble(res.results)
